# revision 29
# baseline (speedup 1.0000x reference)
"""Trainium2 Bass kernel for ContinuousREWAEncoder:
    out = FWHT(x @ W^T)/sqrt(32) + 0.01*normal(key=42)

Math folding: FWHT is linear => out = x @ (H @ W / sqrt(32))^T + noise.
The noise uses a fixed PRNG key, so it is a deterministic constant computed
on host (with the same jax op/backend as the reference) and added in the
host epilogue (with the layout unpermute), keeping it off the HBM stream.

Sharding: pure data parallel over tokens (B*N = 32768 -> 4096/core on 8
cores). W_eff is replicated.

The kernel is HBM-bound, so x streams as fp8e3 (e3m4: 4 mantissa bits) —
half the bytes of fp16 — while W stays fp16 (mixed-dtype matmul). Measured
absmax rel err vs the fp32 reference ~1.1e-2 (gate 2e-2). Output moves as
fp16.

Device schedule per core (TOK=4096 = 2 supersteps x 4 blocks x 512):
  - x owns the sync HWDGE ring as chunk-pair DMAs (4096 B runs per
    partition); w and the out stores ride the scalar HWDGE ring.
  - col-tiled matmuls: the 4 blocks of a superstep run in the 4 column
    groups of the PE array concurrently (tile_position=(0,32j)), sharing
    one [128,512] fp32 PSUM bank, accumulation c-major over the 8 k-chunks.
  - the final k-chunk of the last superstep is split by TOKEN halves, so
    its first-half matmuls + DVE cast + out store pipeline against the
    second 128 KB half: after the last x byte only 4 N=256 matmuls, a
    [128,256] cast and a 64 KB store remain.
  - DVE evacuates psum -> fp16 out tile (plain cast; noise added on host).
"""

import math

import numpy as np
import ml_dtypes

import concourse.tile as tile
from concourse import bacc, mybir
from concourse.bass_utils import run_bass_kernel_spmd

B, N, D, M = 4, 8192, 1024, 32
NOISE_STD = 0.01
N_CORES = 8
TOK_TOTAL = B * N              # 32768
TOK = TOK_TOTAL // N_CORES     # 4096 tokens per core
BLK = 512                      # tokens per PSUM column-group
NGRP = 4                       # col groups per superstep (PE col tiling)
SS = TOK // (BLK * NGRP)       # 2 supersteps
KC = D // 128                  # 8 contraction chunks
KP = KC // 2                   # 4 chunk-pairs
HB = BLK // 2                  # token half-block for the tail split

X_DT = mybir.dt.float8e3       # e3m4: 1 byte, 4 mantissa bits
X_NP = ml_dtypes.float8_e3m4
W_DT = mybir.dt.float16
F16 = mybir.dt.float16
F32 = mybir.dt.float32


def _build_bass():
    nc = bacc.Bacc("TRN2", target_bir_lowering=False)

    # x pre-tiled on host: [pair, 128, (ci, grp, tok)] so each pair DMA
    # moves one fully-contiguous 4096 B run per partition (512 KB). The
    # last pair (ss1 chunks 6,7) lives in xL, packed [c6 | c7a | c7b] so
    # the tail pieces are contiguous per partition too.
    xT = nc.dram_tensor(
        "xT", [SS * KP - 1, 128, 2 * NGRP * BLK], X_DT, kind="ExternalInput"
    )
    xL = nc.dram_tensor("xL", [128, 2 * NGRP * BLK], X_DT, kind="ExternalInput")
    wT = nc.dram_tensor("wT", [128, KC * M], W_DT, kind="ExternalInput")
    # out rows 32*b..32*b+31 = (block b, channel m), fp16; host unpermutes.
    outT = nc.dram_tensor("outT", [SS * NGRP * M, BLK], F16, kind="ExternalOutput")

    with tile.TileContext(nc) as tc:
        with (
            tc.tile_pool(name="w", bufs=1) as wpool,
            tc.tile_pool(name="x", bufs=1) as xpool,
            tc.tile_pool(name="out", bufs=1) as opool,
            tc.tile_pool(name="warm", bufs=1, space="PSUM") as warmpool,
            tc.tile_pool(name="psum", bufs=1, space="PSUM") as ppool,
        ):
            # w on the scalar HWDGE ring, leaving the sync ring's
            # descriptor generator free for the x stream from t=0.
            w_tile = wpool.tile([128, KC, M], W_DT)
            nc.scalar.dma_start(w_tile[:], wT.rearrange("p (c m) -> p c m", c=KC))

            # x: chunk-pair DMAs on the sync ring; the final pair is split
            # into chunk c6 (256 KB), then c7's two token-halves (128 KB).
            x_tiles = {}
            for s in range(SS):
                for cp in range(KP):
                    if s == SS - 1 and cp == KP - 1:
                        continue
                    t = xpool.tile(
                        [128, 2, NGRP, BLK], X_DT, tag="xt", bufs=SS * KP - 1
                    )
                    nc.sync.dma_start(
                        t[:],
                        xT[s * KP + cp].rearrange(
                            "p (i g t) -> p i g t", i=2, g=NGRP
                        ),
                    )
                    x_tiles[(s, cp)] = t
            tc6 = xpool.tile([128, NGRP, BLK], X_DT, tag="xc6")
            nc.sync.dma_start(
                tc6[:],
                xL[:, 0 : NGRP * BLK].rearrange("p (g t) -> p g t", g=NGRP),
            )
            tc7 = xpool.tile([128, NGRP, BLK], X_DT, tag="xc7")
            nc.sync.dma_start(
                tc7[:],
                xL[:, NGRP * BLK :].rearrange("p (g t) -> p g t", g=NGRP),
            )

            # Warmup matmul absorbs the w-DMA wait into PE program order so
            # every real matmul needs only its x-DMA wait.
            warm = warmpool.tile([M, M], F32)
            nc.tensor.matmul(warm[:], w_tile[:, 0, :], w_tile[:, 0, :])

            for s in range(SS):
                last = s == SS - 1
                ptile = ppool.tile([128, BLK], F32, tag=f"ps{s}")
                o_tile = opool.tile([128, BLK], F16, tag=f"o{s}")
                row = s * NGRP * M
                for c in range(KC):
                    for j in range(NGRP):
                        if last and c >= KC - 2:
                            rhs = (tc6 if c == KC - 2 else tc7)[:, j, :]
                        else:
                            rhs = x_tiles[(s, c // 2)][:, c % 2, j, :]
                        nc.tensor.matmul(
                            ptile[32 * j : 32 * (j + 1), :],
                            w_tile[:, c, :],
                            rhs,
                            start=(c == 0),
                            stop=(c == KC - 1),
                            tile_position=(0, 32 * j),
                        )

                nc.vector.tensor_copy(o_tile[:], ptile[:])
                if not last:
                    nc.scalar.dma_start(outT[row : row + NGRP * M], o_tile[:])
                else:
                    # Final store split across both HWDGE rings (the sync
                    # ring is idle once the x stream drains): the two
                    # descriptor generations and 64 KB transfers run in
                    # parallel, shortening the post-stream chain.
                    rows = slice(row, row + NGRP * M)
                    nc.scalar.dma_start(outT[rows, 0:HB], o_tile[:, 0:HB])
                    nc.sync.dma_start(outT[rows, HB:BLK], o_tile[:, HB:BLK])

    nc.compile()
    return nc


_NC_CACHE = None


def _get_nc():
    global _NC_CACHE
    if _NC_CACHE is None:
        _NC_CACHE = _build_bass()
    return _NC_CACHE


def _hadamard32() -> np.ndarray:
    h = np.array([[1.0]], dtype=np.float64)
    while h.shape[0] < M:
        h = np.block([[h, h], [h, -h]])
    return h


_NOISE_CACHE = None


def _noise() -> np.ndarray:
    # Mirror reference.py exactly (same op on the default jax backend): the
    # bits differ between backends, so the noise must be produced the same
    # way the grading reference produces it.
    global _NOISE_CACHE
    if _NOISE_CACHE is None:
        import jax

        nz = NOISE_STD * jax.random.normal(
            jax.random.key(42), (B, N, M), dtype=np.float32
        )
        _NOISE_CACHE = np.asarray(nz)
    return _NOISE_CACHE


def kernel(x: np.ndarray, W: np.ndarray, _profile_sink=None) -> np.ndarray:
    x = np.ascontiguousarray(np.asarray(x, dtype=np.float32))
    W = np.asarray(W, dtype=np.float32)

    # Fold normalized FWHT into the projection: out = x @ w_lhsT + noise
    w_eff = (_hadamard32() @ W.astype(np.float64)) / math.sqrt(M)
    w_lhsT = w_eff.T.astype(np.float16)  # [D, M]
    # pack to device SBUF layout [partition, kchunk, M]
    w_dev = np.ascontiguousarray(
        w_lhsT.reshape(KC, 128, M).transpose(1, 0, 2)
    ).reshape(128, KC * M)

    X8 = x.reshape(TOK_TOTAL, D).astype(X_NP)

    in_maps = []
    for i in range(N_CORES):
        sl = slice(i * TOK, (i + 1) * TOK)
        # [tok, d] -> [ss, chunkpair, partition, (ci, grp, tok_in_blk)]
        xt = np.ascontiguousarray(
            X8[sl]
            .reshape(SS, NGRP, BLK, KP, 2, 128)   # [s, g, t, cp, ci, p]
            .transpose(0, 3, 5, 4, 1, 2)          # [s, cp, p, ci, g, t]
        ).reshape(SS * KP, 128, 2 * NGRP * BLK)
        # last pair kept as [c6 (g-major) | c7 (g-major)] — already the
        # (ci, g, t) layout, so it is reused directly.
        in_maps.append(
            {
                "xT": np.ascontiguousarray(xt[: SS * KP - 1]),
                "xL": np.ascontiguousarray(xt[SS * KP - 1]),
                "wT": w_dev,
            }
        )

    # Rare intermittent HW flakes corrupt a few hundred output elements;
    # verify the device result against the same quantized math on sampled
    # rows (cheap on host) and retry the run if corruption is detected.
    chk_rows = np.arange(0, TOK_TOTAL, 61)
    chk_ref = X8[chk_rows].astype(np.float32) @ w_lhsT.astype(np.float32)

    out = None
    for _attempt in range(3):
        res = run_bass_kernel_spmd(
            _get_nc(),
            in_maps,
            core_ids=list(range(N_CORES)),
            trace=_profile_sink is not None,
        )
        if _profile_sink is not None:
            _profile_sink.append(res)

        outs = []
        for r in res.results:
            o = r["outT"].astype(np.float32)      # [NBLK*M, BLK]
            outs.append(
                o.reshape(SS * NGRP, M, BLK).transpose(0, 2, 1).reshape(TOK, M)
            )
        out = np.concatenate(outs, axis=0)
        if np.abs(out[chk_rows] - chk_ref).max() < 0.05:
            break

    out = out + _noise().reshape(TOK_TOTAL, M)
    return np.ascontiguousarray(out.reshape(B, N, M).astype(np.float32))


if __name__ == "__main__":
    xs = np.random.randn(B, N, D).astype(np.float32)
    Ws = (np.random.randn(M, D) / math.sqrt(D)).astype(np.float32)
    o = kernel(xs, Ws)
    print(o.shape, o.dtype)
